# revision 1
# baseline (speedup 1.0000x reference)
"""Trainium2 Bass kernel for a LoRA-MoE layer (gate top-2 softmax routing +
dense base linear + per-expert low-rank adapters), SPMD across 8 NeuronCores.

Math (per token t):
    logits = x @ gate_w.T                      # [E]
    top-2 softmax over logits -> dense w[E] (0 for non-selected)
    out = x @ base_w.T + base_b
        + SCALING * sum_e w[e] * (x @ lora_A[e].T) @ lora_B[e].T

Key identity used: with w folded into the rank-space activations,
    lora_out = (low * w_rep) @ B_all.T,  low = x @ A_all.T   (A_all: [E*R, D])
so the whole MoE-LoRA is two dense matmuls + tiny gating vector math.

Sharding: 4-way over tokens x 2-way over out-features (8 cores, no
collectives).  Per core: T=1024 tokens, TO=2048 out features.

Layout per core (everything "transposed", contraction dim on partitions):
    out.T[o, t] = sum_d W[o, d] * x.T[d, t]    (x.T moving, W tiles stationary)
"""

import numpy as np

import concourse.bass as bass
import concourse.bass_isa as bass_isa
import concourse.mybir as mybir
import concourse.tile as tile
from concourse import bacc
from concourse.bass_utils import run_bass_kernel_spmd

F32 = mybir.dt.float32
F32R = mybir.dt.float32r

# Problem constants
B, S, D, O = 2, 2048, 4096, 4096
E, R = 8, 16
ER = E * R  # 128
SCALING = 32.0 / 16.0

# Sharding: 4 token groups x 2 out-feature groups
N_CORES = 8
TG, OG = 4, 2
T = (B * S) // TG       # 1024 tokens per core
TO = O // OG            # 2048 out features per core
KT = D // 128           # 32 contraction tiles
OTN = TO // 128         # 16 out tiles per core
CH = 2                  # token chunks of 512
CHW = T // CH           # 512


def build_body(nc, tc, tensors, mm_dt=F32R, gate_dt=F32R):
    xT, wT, aT, gT, bT, bias2, Rm, out = tensors
    AX_C = mybir.AxisListType.C
    OP = mybir.AluOpType

    with (
        tc.tile_pool(name="xp", bufs=KT) as xp,
        tc.tile_pool(name="wp", bufs=4) as wp,
        tc.tile_pool(name="cst", bufs=1) as cst,
        tc.tile_pool(name="apl", bufs=3) as apl,
        tc.tile_pool(name="gw", bufs=1) as gw,
        tc.tile_pool(name="outp", bufs=2) as outp,
        tc.tile_pool(name="psA", bufs=2, space="PSUM") as psA,
        tc.tile_pool(name="psB", bufs=4, space="PSUM") as psB,
    ):
        # ---- constants ----
        bT_sb = cst.tile([ER, TO], F32R)
        nc.gpsimd.dma_start(out=bT_sb[:], in_=bT[:].bitcast(F32R))
        bias_sb = cst.tile([128, OTN], F32)
        nc.gpsimd.dma_start(out=bias_sb[:], in_=bias2[:])
        Rm_sb = cst.tile([E, ER], F32R)
        nc.gpsimd.dma_start(out=Rm_sb[:], in_=Rm[:].bitcast(F32R))

        # ---- resident x.T tiles ----
        x_tiles = []
        for k in range(KT):
            xk = xp.tile([128, T], F32R, tag="x", name=f"x{k}")
            nc.sync.dma_start(out=xk[:], in_=xT[:, k, :].bitcast(F32R))
            x_tiles.append(xk)

        # ---- phase A: low.T = A_all.T^T @ x.T ; gate.T = g^T @ x.T ----
        low_ps = [psA.tile([ER, CHW], F32, tag="low", name=f"lowps{c}") for c in range(CH)]
        gate_ps = [psA.tile([E, CHW], F32, tag="gate", name=f"gateps{c}") for c in range(CH)]
        for k in range(KT):
            ak = apl.tile([128, ER], F32R, tag="a", name=f"a{k}")
            nc.gpsimd.dma_start(out=ak[:], in_=aT[:, k, :].bitcast(F32R))
            gk = apl.tile([128, E], F32R, tag="g", name=f"g{k}")
            nc.gpsimd.dma_start(out=gk[:], in_=gT[:, k, :].bitcast(F32R))
            for c in range(CH):
                rhs = x_tiles[k][:, c * CHW:(c + 1) * CHW]
                nc.tensor.matmul(low_ps[c][:], lhsT=ak[:], rhs=rhs,
                                 start=(k == 0), stop=(k == KT - 1))
                nc.tensor.matmul(gate_ps[c][:], lhsT=gk[:], rhs=rhs,
                                 start=(k == 0), stop=(k == KT - 1))

        # ---- gating math in [E, t] layout, per 512-token chunk ----
        lowT_sb = gw.tile([ER, T], F32R, tag="lowT")
        for c in range(CH):
            cs = slice(c * CHW, (c + 1) * CHW)
            g_sb = gw.tile([E, CHW], F32, tag="gsb", name=f"gsb{c}")
            nc.vector.tensor_copy(g_sb[:], gate_ps[c][:])
            m1b = gw.tile([E, CHW], F32, tag="m1b", name=f"m1b{c}")
            nc.gpsimd.partition_all_reduce(m1b[:], g_sb[:], channels=E,
                                           reduce_op=bass_isa.ReduceOp.max)
            eq = gw.tile([E, CHW], F32, tag="tmp", bufs=3, name=f"eq{c}")
            nc.vector.tensor_tensor(eq[:], g_sb[:], m1b[:], op=OP.is_equal)
            gm = gw.tile([E, CHW], F32, tag="tmp", bufs=3, name=f"gm{c}")
            nc.vector.scalar_tensor_tensor(gm[:], in0=eq[:], scalar=-1e30, in1=g_sb[:],
                                           op0=OP.mult, op1=OP.add)
            m2b = gw.tile([E, CHW], F32, tag="m2b", name=f"m2b{c}")
            nc.gpsimd.partition_all_reduce(m2b[:], gm[:], channels=E,
                                           reduce_op=bass_isa.ReduceOp.max)
            diff = gw.tile([E, CHW], F32, tag="tmp", bufs=3, name=f"diff{c}")
            nc.vector.tensor_sub(diff[:], g_sb[:], m1b[:])
            ex = gw.tile([E, CHW], F32, tag="ex", name=f"ex{c}")
            nc.scalar.activation(ex[:], diff[:], mybir.ActivationFunctionType.Exp)
            mask = gw.tile([E, CHW], F32, tag="tmp", bufs=3, name=f"mask{c}")
            nc.vector.tensor_tensor(mask[:], g_sb[:], m2b[:], op=OP.is_ge)
            wn = gw.tile([E, CHW], F32, tag="wn", name=f"wn{c}")
            nc.vector.tensor_mul(wn[:], ex[:], mask[:])
            # denominator 1 + exp(m2 - m1), computed broadcast on all 8 rows
            dmb = gw.tile([E, CHW], F32, tag="tmp", bufs=3, name=f"dmb{c}")
            nc.vector.tensor_sub(dmb[:], m2b[:], m1b[:])
            edb = gw.tile([E, CHW], F32, tag="edb", name=f"edb{c}")
            nc.scalar.activation(edb[:], dmb[:], mybir.ActivationFunctionType.Exp)
            denb = gw.tile([E, CHW], F32, tag="tmp", bufs=3, name=f"denb{c}")
            nc.vector.tensor_scalar_add(denb[:], edb[:], 1.0)
            recb = gw.tile([E, CHW], F32, tag="recb", name=f"recb{c}")
            nc.vector.reciprocal(recb[:], denb[:])
            wsc = gw.tile([E, CHW], F32R, tag="wsc", name=f"wsc{c}")
            nc.vector.scalar_tensor_tensor(wsc[:], in0=wn[:], scalar=SCALING, in1=recb[:],
                                           op0=OP.mult, op1=OP.mult)
            # replicate each expert weight over its 16 ranks via tiny matmul
            wrep_ps = psA.tile([ER, CHW], F32, tag="gate", name=f"wrep{c}")
            nc.tensor.matmul(wrep_ps[:], lhsT=Rm_sb[:], rhs=wsc[:],
                             start=True, stop=True)
            # low_w.T = low.T * w_rep  (copy wrep to SBUF first: DVE has a
            # single PSUM read port, two-PSUM-operand tensor_tensor is illegal)
            wrep_sb = gw.tile([ER, CHW], F32, tag="wrepsb", name=f"wrepsb{c}")
            nc.scalar.copy(wrep_sb[:], wrep_ps[:])
            nc.vector.tensor_tensor(lowT_sb[:, cs], low_ps[c][:], wrep_sb[:], op=OP.mult)

        # ---- phase B: out.T tiles = W-tile^T @ x.T  (+ B-tile^T @ low_w.T) ----
        for ot in range(OTN):
            wtiles = []
            for q in range(4):
                wq = wp.tile([128, 8, 128], F32R, tag="w", name=f"w{ot}_{q}")
                nc.scalar.dma_start(out=wq[:], in_=wT[:, ot, q * 8:(q + 1) * 8, :].bitcast(F32R))
                wtiles.append(wq)
            pb = [psB.tile([128, CHW], F32, tag="pb", name=f"pb{ot}_{c}") for c in range(CH)]
            for k in range(KT):
                wk = wtiles[k // 8][:, k % 8, :]
                for c in range(CH):
                    nc.tensor.matmul(pb[c][:], lhsT=wk,
                                     rhs=x_tiles[k][:, c * CHW:(c + 1) * CHW],
                                     start=(k == 0), stop=False)
            for c in range(CH):
                nc.tensor.matmul(pb[c][:], lhsT=bT_sb[:, ot * 128:(ot + 1) * 128],
                                 rhs=lowT_sb[:, c * CHW:(c + 1) * CHW],
                                 start=False, stop=True)
            o_sb = outp.tile([128, T], F32, tag="o", name=f"o{ot}")
            for c in range(CH):
                nc.vector.tensor_scalar(o_sb[:, c * CHW:(c + 1) * CHW], pb[c][:],
                                        scalar1=bias_sb[:, ot:ot + 1], scalar2=None,
                                        op0=OP.add)
            nc.gpsimd.dma_start(out=out[:, ot, :], in_=o_sb[:])


def build_module(mm_dt=F32R, gate_dt=F32R, debug=False):
    nc = bacc.Bacc("TRN2", target_bir_lowering=False, debug=debug)
    xT = nc.dram_tensor("xT", [128, KT, T], F32, kind="ExternalInput")
    wT = nc.dram_tensor("wT", [128, OTN, KT, 128], F32, kind="ExternalInput")
    aT = nc.dram_tensor("aT", [128, KT, ER], F32, kind="ExternalInput")
    gT = nc.dram_tensor("gT", [128, KT, E], F32, kind="ExternalInput")
    bT = nc.dram_tensor("bT", [ER, TO], F32, kind="ExternalInput")
    bias2 = nc.dram_tensor("bias2", [128, OTN], F32, kind="ExternalInput")
    Rm = nc.dram_tensor("Rm", [E, ER], F32, kind="ExternalInput")
    out = nc.dram_tensor("out", [128, OTN, T], F32, kind="ExternalOutput")
    with tile.TileContext(nc) as tc:
        build_body(nc, tc, (xT, wT, aT, gT, bT, bias2, Rm, out),
                   mm_dt=mm_dt, gate_dt=gate_dt)
    nc.compile()
    return nc


def shard_inputs(x, gate_w, base_w, base_b, lora_A, lora_B):
    """FULL inputs -> list of 8 per-core input maps (host-side, free)."""
    x = np.asarray(x, dtype=np.float32)
    gate_w = np.asarray(gate_w, dtype=np.float32)
    base_w = np.asarray(base_w, dtype=np.float32)
    base_b = np.asarray(base_b, dtype=np.float32)
    lora_A = np.asarray(lora_A, dtype=np.float32)
    lora_B = np.asarray(lora_B, dtype=np.float32)

    xf = x.reshape(B * S, D)
    # replicated smalls
    gT = np.ascontiguousarray(gate_w.T.reshape(KT, 128, E).transpose(1, 0, 2))
    A_flat = lora_A.reshape(ER, D)
    aT = np.ascontiguousarray(A_flat.T.reshape(KT, 128, ER).transpose(1, 0, 2))
    B_flat = lora_B.transpose(0, 2, 1).reshape(ER, O)   # [er, o]
    Rm = np.repeat(np.eye(E, dtype=np.float32), R, axis=1)  # [E, ER]

    in_maps = []
    for c in range(N_CORES):
        tg, og = c // OG, c % OG
        x_c = xf[tg * T:(tg + 1) * T]                       # [T, D]
        xT = np.ascontiguousarray(x_c.T.reshape(KT, 128, T).transpose(1, 0, 2))
        w_c = base_w[og * TO:(og + 1) * TO]                 # [TO, D]
        wT = np.ascontiguousarray(
            w_c.reshape(OTN, 128, KT, 128).transpose(3, 0, 2, 1))
        bT = np.ascontiguousarray(B_flat[:, og * TO:(og + 1) * TO])
        bias2 = np.ascontiguousarray(base_b[og * TO:(og + 1) * TO].reshape(OTN, 128).T)
        in_maps.append({"xT": xT, "wT": wT, "aT": aT, "gT": gT,
                        "bT": bT, "bias2": bias2, "Rm": Rm})
    return in_maps


def gather_outputs(results):
    """list of 8 per-core result maps -> FULL output [B, S, O]."""
    full = np.empty((B * S, O), dtype=np.float32)
    for c in range(N_CORES):
        tg, og = c // OG, c % OG
        oc = results[c]["out"]                              # [128, OTN, T]
        full[tg * T:(tg + 1) * T, og * TO:(og + 1) * TO] = \
            oc.transpose(2, 1, 0).reshape(T, TO)
    return full.reshape(B, S, O)


_NC_CACHE = {}


def _get_module(mm_dt=F32R, gate_dt=F32R):
    key = (mm_dt, gate_dt)
    if key not in _NC_CACHE:
        _NC_CACHE[key] = build_module(mm_dt=mm_dt, gate_dt=gate_dt)
    return _NC_CACHE[key]


def run_sharded(in_maps, mm_dt=F32R, gate_dt=F32R, **run_kwargs):
    nc = _get_module(mm_dt=mm_dt, gate_dt=gate_dt)
    return run_bass_kernel_spmd(nc, in_maps, list(range(N_CORES)), **run_kwargs)


def kernel(x, gate_w, base_w, base_b, lora_A, lora_B):
    in_maps = shard_inputs(x, gate_w, base_w, base_b, lora_A, lora_B)
    res = run_sharded(in_maps)
    return gather_outputs(res.results)



# revision 3
# speedup vs baseline: 1.2485x; 1.2485x over previous
"""Trainium2 Bass kernel for a LoRA-MoE layer (gate top-2 softmax routing +
dense base linear + per-expert low-rank adapters), SPMD across 8 NeuronCores.

Math (per token t):
    logits = x @ gate_w.T                      # [E]
    top-2 softmax over logits -> dense w[E] (0 for non-selected)
    out = x @ base_w.T + base_b
        + SCALING * sum_e w[e] * (x @ lora_A[e].T) @ lora_B[e].T

Key identity used: with w folded into the rank-space activations,
    lora_out = (low * w_rep) @ B_all.T,  low = x @ A_all.T   (A_all: [E*R, D])
so the whole MoE-LoRA is two dense matmuls + tiny gating vector math.
Top-2 softmax via sigmoid: w(g) = sigmoid(2g - m1 - m2) for g in {m1, m2}.

Sharding: 8-way over tokens (T=512/core), base W replicated (full O=4096
per core).  All matmul operands bf16 (PSUM accumulates fp32); output bf16,
host converts to fp32.  No collectives.

Layout per core (contraction dim on partitions):
    out.T[o, t] = sum_d W[o, d] * x.T[d, t]    (x.T moving, W tiles stationary)

Schedule: phase A (low+gate, 64 matmuls) -> gating math on DVE/gpsimd/ACT
overlapped under the first base-W chains; each out-tile's lora-B matmul is
appended to its (still open) PSUM chain 3 out-tiles later so the gating
latency never stalls the PE.
"""

import numpy as np
import ml_dtypes

import concourse.bass as bass
import concourse.bass_isa as bass_isa
import concourse.mybir as mybir
import concourse.tile as tile
from concourse import bacc
from concourse.bass_utils import run_bass_kernel_spmd

F32 = mybir.dt.float32
BF16 = mybir.dt.bfloat16
NP_BF16 = ml_dtypes.bfloat16

# Problem constants
B, S, D, O = 2, 2048, 4096, 4096
E, R = 8, 16
ER = E * R  # 128
SCALING = 32.0 / 16.0

# Sharding: 8 token groups, base W replicated
N_CORES = 8
T = (B * S) // N_CORES  # 512 tokens per core
KT = D // 128           # 32 contraction tiles
OTN = O // 128          # 32 out tiles per core
XCH = 4                 # x loaded in 4 chunked DMAs of 8 k-tiles each
KPC = KT // XCH         # 8
LORA_DELAY = 3          # out-tiles between base chain and its lora-B append


def build_body(nc, tc, tensors):
    xT, wT, aT, gT, bT, bias2, Rm, out = tensors
    OP = mybir.AluOpType

    with (
        tc.tile_pool(name="xp", bufs=XCH) as xp,
        tc.tile_pool(name="wp", bufs=3) as wp,
        tc.tile_pool(name="cst", bufs=1) as cst,
        tc.tile_pool(name="gw", bufs=1) as gw,
        tc.tile_pool(name="outp", bufs=4) as outp,
        tc.tile_pool(name="psA", bufs=1, space="PSUM") as psA,
        tc.tile_pool(name="psB", bufs=4, space="PSUM") as psB,
    ):
        # ---- small constants (gpsimd queue; phase-A inputs first) ----
        aT_sb = cst.tile([128, KT, 128], BF16)
        nc.gpsimd.dma_start(out=aT_sb[:], in_=aT[:])
        gT_sb = cst.tile([128, KT, E], BF16)
        nc.gpsimd.dma_start(out=gT_sb[:], in_=gT[:])
        Rm_sb = cst.tile([E, ER], BF16)
        nc.gpsimd.dma_start(out=Rm_sb[:], in_=Rm[:])
        bias_sb = cst.tile([128, OTN], F32)
        nc.gpsimd.dma_start(out=bias_sb[:], in_=bias2[:])
        bT_sb = cst.tile([ER, O], BF16)
        nc.gpsimd.dma_start(out=bT_sb[:], in_=bT[:])

        # ---- x.T chunks (sync queue), then first W tiles on the same
        # queue so x packets drain ahead of the W stream ----
        x_ch = []
        for c in range(XCH):
            xc = xp.tile([128, KPC, T], BF16, tag="x", name=f"x{c}")
            nc.sync.dma_start(out=xc[:], in_=xT[:, c * KPC:(c + 1) * KPC, :])
            x_ch.append(xc)

        w_tiles = {}
        for ot in range(3):
            wq = wp.tile([128, KT, 128], BF16, tag="w", name=f"w{ot}")
            nc.sync.dma_start(out=wq[:], in_=wT[:, ot])
            w_tiles[ot] = wq

        def x_k(k):
            return x_ch[k // KPC][:, k % KPC, :]

        # ---- phase A: low.T = A_all.T^T @ x.T ; gate.T = g^T @ x.T ----
        low_ps = psA.tile([ER, T], F32, tag="low")
        gate_ps = psA.tile([E, T], F32, tag="gate")
        for k in range(KT):
            nc.tensor.matmul(low_ps[:], lhsT=aT_sb[:, k, :], rhs=x_k(k),
                             start=(k == 0), stop=(k == KT - 1))
            nc.tensor.matmul(gate_ps[:], lhsT=gT_sb[:, k, :], rhs=x_k(k),
                             start=(k == 0), stop=(k == KT - 1))

        # ---- gating math in [E, t] layout (latency hidden under phase B) --
        g_sb = gw.tile([E, T], F32, tag="gsb")
        nc.vector.tensor_copy(g_sb[:], gate_ps[:])
        m1b = gw.tile([E, T], F32, tag="m1b")
        nc.gpsimd.partition_all_reduce(m1b[:], g_sb[:], channels=E,
                                       reduce_op=bass_isa.ReduceOp.max)
        eq = gw.tile([E, T], F32, tag="eq")
        nc.vector.tensor_tensor(eq[:], g_sb[:], m1b[:], op=OP.is_equal)
        gm = gw.tile([E, T], F32, tag="gm")
        nc.vector.scalar_tensor_tensor(gm[:], in0=eq[:], scalar=-1e30, in1=g_sb[:],
                                       op0=OP.mult, op1=OP.add)
        m2b = gw.tile([E, T], F32, tag="m2b")
        nc.gpsimd.partition_all_reduce(m2b[:], gm[:], channels=E,
                                       reduce_op=bass_isa.ReduceOp.max)
        # top-2 softmax weights: w = sigmoid(2g - m1 - m2) on the two
        # selected rows (exact: sigmoid(m1-m2) and sigmoid(m2-m1))
        ssum = gw.tile([E, T], F32, tag="ssum")
        nc.vector.tensor_tensor(ssum[:], m1b[:], m2b[:], op=OP.add)
        dd = gw.tile([E, T], F32, tag="dd")
        nc.vector.scalar_tensor_tensor(dd[:], in0=g_sb[:], scalar=2.0, in1=ssum[:],
                                       op0=OP.mult, op1=OP.subtract)
        sg = gw.tile([E, T], F32, tag="sg")
        nc.scalar.activation(sg[:], dd[:], mybir.ActivationFunctionType.Sigmoid)
        mask = gw.tile([E, T], F32, tag="mask")
        nc.vector.tensor_tensor(mask[:], g_sb[:], m2b[:], op=OP.is_ge)
        wsc = gw.tile([E, T], BF16, tag="wsc")
        nc.vector.scalar_tensor_tensor(wsc[:], in0=sg[:], scalar=SCALING, in1=mask[:],
                                       op0=OP.mult, op1=OP.mult)

        # ---- phase B: out.T[ot] = W[ot]^T @ x.T (+ B[ot]^T @ low_w.T) ----
        open_chains = []  # (ot, psum tile) awaiting their lora-B append

        def finish(ot, pb):
            nc.tensor.matmul(pb[:], lhsT=bT_sb[:, ot * 128:(ot + 1) * 128],
                             rhs=lowT_sb[:], start=False, stop=True)
            o_sb = outp.tile([128, T], BF16, tag="o", name=f"o{ot}")
            nc.vector.tensor_scalar(o_sb[:], pb[:],
                                    scalar1=bias_sb[:, ot:ot + 1], scalar2=None,
                                    op0=OP.add)
            nc.gpsimd.dma_start(out=out[:, ot, :], in_=o_sb[:])

        lowT_sb = gw.tile([ER, T], BF16, tag="lowT")
        for ot in range(OTN):
            if ot in w_tiles:
                wq = w_tiles.pop(ot)
            else:
                wq = wp.tile([128, KT, 128], BF16, tag="w", name=f"w{ot}")
                nc.scalar.dma_start(out=wq[:], in_=wT[:, ot])
            pb = psB.tile([128, T], F32, tag="pb", name=f"pb{ot}")
            for k in range(KT):
                nc.tensor.matmul(pb[:], lhsT=wq[:, k, :], rhs=x_k(k),
                                 start=(k == 0), stop=False)
            if ot == 0:
                # replicate expert weights over their 16 ranks via tiny
                # matmul, then fold into the rank-space activations
                wrep_ps = psA.tile([ER, T], F32, tag="wrep")
                nc.tensor.matmul(wrep_ps[:], lhsT=Rm_sb[:], rhs=wsc[:],
                                 start=True, stop=True)
                wrep_sb = gw.tile([ER, T], F32, tag="wrepsb")
                nc.scalar.copy(wrep_sb[:], wrep_ps[:])
                nc.vector.tensor_tensor(lowT_sb[:], low_ps[:], wrep_sb[:],
                                        op=OP.mult)
            open_chains.append((ot, pb))
            if len(open_chains) > LORA_DELAY:
                finish(*open_chains.pop(0))
        for ot, pb in open_chains:
            finish(ot, pb)


def build_module(debug=False):
    nc = bacc.Bacc("TRN2", target_bir_lowering=False, debug=debug)
    xT = nc.dram_tensor("xT", [128, KT, T], BF16, kind="ExternalInput")
    wT = nc.dram_tensor("wT", [128, OTN, KT, 128], BF16, kind="ExternalInput")
    aT = nc.dram_tensor("aT", [128, KT, ER], BF16, kind="ExternalInput")
    gT = nc.dram_tensor("gT", [128, KT, E], BF16, kind="ExternalInput")
    bT = nc.dram_tensor("bT", [ER, O], BF16, kind="ExternalInput")
    bias2 = nc.dram_tensor("bias2", [128, OTN], F32, kind="ExternalInput")
    Rm = nc.dram_tensor("Rm", [E, ER], BF16, kind="ExternalInput")
    out = nc.dram_tensor("out", [128, OTN, T], BF16, kind="ExternalOutput")
    with tile.TileContext(nc) as tc:
        build_body(nc, tc, (xT, wT, aT, gT, bT, bias2, Rm, out))
    nc.compile()
    return nc


def shard_inputs(x, gate_w, base_w, base_b, lora_A, lora_B):
    """FULL inputs -> list of 8 per-core input maps (host-side, free)."""
    x = np.asarray(x, dtype=np.float32)
    gate_w = np.asarray(gate_w, dtype=np.float32)
    base_w = np.asarray(base_w, dtype=np.float32)
    base_b = np.asarray(base_b, dtype=np.float32)
    lora_A = np.asarray(lora_A, dtype=np.float32)
    lora_B = np.asarray(lora_B, dtype=np.float32)

    xf = x.reshape(B * S, D)
    # replicated tensors (shared across cores)
    gT = np.ascontiguousarray(
        gate_w.T.reshape(KT, 128, E).transpose(1, 0, 2)).astype(NP_BF16)
    A_flat = lora_A.reshape(ER, D)
    aT = np.ascontiguousarray(
        A_flat.T.reshape(KT, 128, ER).transpose(1, 0, 2)).astype(NP_BF16)
    bT = np.ascontiguousarray(
        lora_B.transpose(0, 2, 1).reshape(ER, O)).astype(NP_BF16)
    Rm = np.repeat(np.eye(E, dtype=np.float32), R, axis=1).astype(NP_BF16)
    wT = np.ascontiguousarray(
        base_w.reshape(OTN, 128, KT, 128).transpose(3, 0, 2, 1)).astype(NP_BF16)
    bias2 = np.ascontiguousarray(base_b.reshape(OTN, 128).T)

    in_maps = []
    for c in range(N_CORES):
        x_c = xf[c * T:(c + 1) * T]                         # [T, D]
        xTc = np.ascontiguousarray(
            x_c.T.reshape(KT, 128, T).transpose(1, 0, 2)).astype(NP_BF16)
        in_maps.append({"xT": xTc, "wT": wT, "aT": aT, "gT": gT,
                        "bT": bT, "bias2": bias2, "Rm": Rm})
    return in_maps


def gather_outputs(results):
    """list of 8 per-core result maps -> FULL output [B, S, O]."""
    full = np.empty((B * S, O), dtype=np.float32)
    for c in range(N_CORES):
        oc = results[c]["out"]                              # [128, OTN, T] bf16
        full[c * T:(c + 1) * T, :] = \
            oc.transpose(2, 1, 0).reshape(T, O).astype(np.float32)
    return full.reshape(B, S, O)


_NC_CACHE = {}


def _get_module():
    if "nc" not in _NC_CACHE:
        _NC_CACHE["nc"] = build_module()
    return _NC_CACHE["nc"]


def run_sharded(in_maps, **run_kwargs):
    nc = _get_module()
    return run_bass_kernel_spmd(nc, in_maps, list(range(N_CORES)), **run_kwargs)


def kernel(x, gate_w, base_w, base_b, lora_A, lora_B):
    in_maps = shard_inputs(x, gate_w, base_w, base_b, lora_A, lora_B)
    res = run_sharded(in_maps)
    return gather_outputs(res.results)


# revision 9
# speedup vs baseline: 1.2766x; 1.0226x over previous
"""Trainium2 Bass kernel for a LoRA-MoE layer (gate top-2 softmax routing +
dense base linear + per-expert low-rank adapters), SPMD across 8 NeuronCores.

Math (per token t):
    logits = x @ gate_w.T                      # [E]
    top-2 softmax over logits -> dense w[E] (0 for non-selected)
    out = x @ base_w.T + base_b
        + SCALING * sum_e w[e] * (x @ lora_A[e].T) @ lora_B[e].T

Key identity used: with w folded into the rank-space activations,
    lora_out = (low * w_rep) @ B_all.T,  low = x @ A_all.T   (A_all: [E*R, D])
so the whole MoE-LoRA is two dense matmuls + tiny gating vector math.
Top-2 softmax via sigmoid: w(g) = sigmoid(2g - m1 - m2) for g in {m1, m2}.

Sharding: 8-way over tokens (T=512/core), base W replicated (full O=4096
per core).  All matmul operands bf16 (PSUM accumulates fp32); output bf16,
host converts to fp32.  No collectives.

Layout per core (contraction dim on partitions):
    out.T[o, t] = sum_d W[o, d] * x.T[d, t]    (x.T moving, W tiles stationary)

Schedule: phase A (low+gate, 64 matmuls) -> gating math on DVE/gpsimd/ACT
overlapped under the first base-W chains; each out-tile's lora-B matmul is
appended to its (still open) PSUM chain 3 out-tiles later so the gating
latency never stalls the PE.
"""

import numpy as np
import ml_dtypes

import concourse.bass as bass
import concourse.bass_isa as bass_isa
import concourse.mybir as mybir
import concourse.tile as tile
from concourse import bacc
from concourse.bass_utils import run_bass_kernel_spmd

F32 = mybir.dt.float32
BF16 = mybir.dt.bfloat16
NP_BF16 = ml_dtypes.bfloat16

# Problem constants
B, S, D, O = 2, 2048, 4096, 4096
E, R = 8, 16
ER = E * R  # 128
SCALING = 32.0 / 16.0

# Sharding: 8 token groups, base W replicated
N_CORES = 8
T = (B * S) // N_CORES  # 512 tokens per core
KT = D // 128           # 32 contraction tiles
OTN = O // 128          # 32 out tiles per core
XSPLIT = (4, 4, 8, 16)  # x chunk sizes in k-tiles (small first for fast start)
ASPLIT = (8, 24)        # aT chunk sizes in k-tiles
W_SYNC = 6              # first W tiles ride the sync queue behind x
W_BUFS = 6              # W prefetch depth


def build_body(nc, tc, tensors):
    xT, wT, aT, gT, bT, bias2, Rm, out = tensors
    OP = mybir.AluOpType

    with (
        tc.tile_pool(name="xp", bufs=1) as xp,
        tc.tile_pool(name="wp", bufs=W_BUFS) as wp,
        tc.tile_pool(name="cst", bufs=1) as cst,
        tc.tile_pool(name="gw", bufs=1) as gw,
        tc.tile_pool(name="outp", bufs=4) as outp,
        tc.tile_pool(name="psA", bufs=1, space="PSUM") as psA,
        tc.tile_pool(name="psB", bufs=4, space="PSUM") as psB,
    ):
        # ---- tiny constants on the (otherwise idle) gpsimd queue ----
        Rm_sb = cst.tile([E, ER], BF16)
        nc.gpsimd.dma_start(out=Rm_sb[:], in_=Rm[:])
        bias_sb = cst.tile([128, OTN], F32)
        nc.gpsimd.dma_start(out=bias_sb[:], in_=bias2[:])

        # ---- everything phase A needs rides the fast sync queue, in
        # consumption order: aT chunk 0, gT, x chunks (small first), the
        # second aT chunk, and the first W tiles snuck in behind x ----
        a_ch, a_base = [], 0
        gT_sb = None
        x_ch, x_base = [], 0
        w_tiles = {}

        ac = cst.tile([128, ASPLIT[0], 128], BF16, name="a0")
        nc.sync.dma_start(out=ac[:], in_=aT[:, :ASPLIT[0], :])
        a_ch.append((0, ac))
        gT_sb = cst.tile([128, KT, E], BF16)
        nc.sync.dma_start(out=gT_sb[:], in_=gT[:])
        for c, nk in enumerate(XSPLIT[:3]):
            xc = xp.tile([128, nk, T], BF16, name=f"x{c}")
            nc.sync.dma_start(out=xc[:], in_=xT[:, x_base:x_base + nk, :])
            x_ch.append((x_base, xc))
            x_base += nk
            if c == 0:
                ac = cst.tile([128, ASPLIT[1], 128], BF16, name="a1")
                nc.sync.dma_start(out=ac[:], in_=aT[:, ASPLIT[0]:, :])
                a_ch.append((ASPLIT[0], ac))
        wq = wp.tile([128, KT, 128], BF16, tag="w", name="w0")
        nc.sync.dma_start(out=wq[:], in_=wT[:, 0])
        w_tiles[0] = wq
        nk = XSPLIT[3]
        xc = xp.tile([128, nk, T], BF16, name="x3")
        nc.sync.dma_start(out=xc[:], in_=xT[:, x_base:x_base + nk, :])
        x_ch.append((x_base, xc))
        for ot in range(1, W_SYNC):
            wq = wp.tile([128, KT, 128], BF16, tag="w", name=f"w{ot}")
            nc.sync.dma_start(out=wq[:], in_=wT[:, ot])
            w_tiles[ot] = wq

        def x_k(k):
            for base, xc in reversed(x_ch):
                if k >= base:
                    return xc[:, k - base, :]

        def a_k(k):
            for base, ac in reversed(a_ch):
                if k >= base:
                    return ac[:, k - base, :]

        # ---- phase A: low.T = A_all.T^T @ x.T ; gate.T = g^T @ x.T ----
        low_ps = psA.tile([ER, T], F32, tag="low")
        gate_ps = psA.tile([E, T], F32, tag="gate")
        for k in range(KT):
            nc.tensor.matmul(low_ps[:], lhsT=a_k(k), rhs=x_k(k),
                             start=(k == 0), stop=(k == KT - 1))
            nc.tensor.matmul(gate_ps[:], lhsT=gT_sb[:, k, :], rhs=x_k(k),
                             start=(k == 0), stop=(k == KT - 1))

        # ---- gating math in [E, t] layout (latency hidden under phase B) --
        g_sb = gw.tile([E, T], F32, tag="gsb")
        nc.vector.tensor_copy(g_sb[:], gate_ps[:])
        m1b = gw.tile([E, T], F32, tag="m1b")
        nc.gpsimd.partition_all_reduce(m1b[:], g_sb[:], channels=E,
                                       reduce_op=bass_isa.ReduceOp.max)
        eq = gw.tile([E, T], F32, tag="eq")
        nc.vector.tensor_tensor(eq[:], g_sb[:], m1b[:], op=OP.is_equal)
        gm = gw.tile([E, T], F32, tag="gm")
        nc.vector.scalar_tensor_tensor(gm[:], in0=eq[:], scalar=-1e30, in1=g_sb[:],
                                       op0=OP.mult, op1=OP.add)
        m2b = gw.tile([E, T], F32, tag="m2b")
        nc.gpsimd.partition_all_reduce(m2b[:], gm[:], channels=E,
                                       reduce_op=bass_isa.ReduceOp.max)
        # lora-B weights: issued here (gpsimd program order) so the 1MB
        # load lands in the post-head DMA lull, not the critical head
        bT_sb = cst.tile([ER, O], BF16)
        nc.gpsimd.dma_start(out=bT_sb[:], in_=bT[:])
        # top-2 softmax weights: w = sigmoid(2g - m1 - m2) on the two
        # selected rows (exact: sigmoid(m1-m2) and sigmoid(m2-m1))
        ssum = gw.tile([E, T], F32, tag="ssum")
        nc.vector.tensor_tensor(ssum[:], m1b[:], m2b[:], op=OP.add)
        dd = gw.tile([E, T], F32, tag="dd")
        nc.vector.scalar_tensor_tensor(dd[:], in0=g_sb[:], scalar=2.0, in1=ssum[:],
                                       op0=OP.mult, op1=OP.subtract)
        sg = gw.tile([E, T], F32, tag="sg")
        nc.scalar.activation(sg[:], dd[:], mybir.ActivationFunctionType.Sigmoid)
        mask = gw.tile([E, T], F32, tag="mask")
        nc.vector.tensor_tensor(mask[:], g_sb[:], m2b[:], op=OP.is_ge)
        wsc = gw.tile([E, T], BF16, tag="wsc")
        nc.vector.scalar_tensor_tensor(wsc[:], in0=sg[:], scalar=SCALING, in1=mask[:],
                                       op0=OP.mult, op1=OP.mult)

        # ---- phase B: out.T[ot] = W[ot]^T @ x.T (+ B[ot]^T @ low_w.T) ----
        open_chains = []  # (ot, psum tile) awaiting their lora-B append

        def finish(ot, pb):
            nc.tensor.matmul(pb[:], lhsT=bT_sb[:, ot * 128:(ot + 1) * 128],
                             rhs=lowT_sb[:], start=False, stop=True)
            o_sb = outp.tile([128, T], BF16, tag="o", name=f"o{ot}")
            nc.vector.tensor_scalar(o_sb[:], pb[:],
                                    scalar1=bias_sb[:, ot:ot + 1], scalar2=None,
                                    op0=OP.add)
            nc.gpsimd.dma_start(out=out[:, ot, :], in_=o_sb[:])

        lowT_sb = gw.tile([ER, T], BF16, tag="lowT")
        for ot in range(OTN):
            if ot in w_tiles:
                wq = w_tiles.pop(ot)
            else:
                wq = wp.tile([128, KT, 128], BF16, tag="w", name=f"w{ot}")
                eng = nc.scalar if ot % 2 == 0 else nc.sync
                eng.dma_start(out=wq[:], in_=wT[:, ot])
            pb = psB.tile([128, T], F32, tag="pb", name=f"pb{ot}")
            for k in range(KT):
                nc.tensor.matmul(pb[:], lhsT=wq[:, k, :], rhs=x_k(k),
                                 start=(k == 0), stop=False)
            if ot == 2:
                # replicate expert weights over their 16 ranks via tiny
                # matmul, then fold into the rank-space activations;
                # emitted late enough that the gating chain is done
                wrep_ps = psA.tile([ER, T], F32, tag="wrep")
                nc.tensor.matmul(wrep_ps[:], lhsT=Rm_sb[:], rhs=wsc[:],
                                 start=True, stop=True)
                wrep_sb = gw.tile([ER, T], F32, tag="wrepsb")
                nc.scalar.copy(wrep_sb[:], wrep_ps[:])
                nc.vector.tensor_tensor(lowT_sb[:], low_ps[:], wrep_sb[:],
                                        op=OP.mult)
            open_chains.append((ot, pb))
            # drain to steady delay-1 (first finish only after the gating
            # fold at ot==2 has had an out-tile of slack)
            while open_chains and ot >= 3 and len(open_chains) > max(1, 3 - (ot - 3)):
                finish(*open_chains.pop(0))
        for ot, pb in open_chains:
            finish(ot, pb)


def build_module(debug=False):
    nc = bacc.Bacc("TRN2", target_bir_lowering=False, debug=debug)
    xT = nc.dram_tensor("xT", [128, KT, T], BF16, kind="ExternalInput")
    wT = nc.dram_tensor("wT", [128, OTN, KT, 128], BF16, kind="ExternalInput")
    aT = nc.dram_tensor("aT", [128, KT, ER], BF16, kind="ExternalInput")
    gT = nc.dram_tensor("gT", [128, KT, E], BF16, kind="ExternalInput")
    bT = nc.dram_tensor("bT", [ER, O], BF16, kind="ExternalInput")
    bias2 = nc.dram_tensor("bias2", [128, OTN], F32, kind="ExternalInput")
    Rm = nc.dram_tensor("Rm", [E, ER], BF16, kind="ExternalInput")
    out = nc.dram_tensor("out", [128, OTN, T], BF16, kind="ExternalOutput")
    with tile.TileContext(nc) as tc:
        build_body(nc, tc, (xT, wT, aT, gT, bT, bias2, Rm, out))
    nc.compile()
    return nc


def shard_inputs(x, gate_w, base_w, base_b, lora_A, lora_B):
    """FULL inputs -> list of 8 per-core input maps (host-side, free)."""
    x = np.asarray(x, dtype=np.float32)
    gate_w = np.asarray(gate_w, dtype=np.float32)
    base_w = np.asarray(base_w, dtype=np.float32)
    base_b = np.asarray(base_b, dtype=np.float32)
    lora_A = np.asarray(lora_A, dtype=np.float32)
    lora_B = np.asarray(lora_B, dtype=np.float32)

    xf = x.reshape(B * S, D)
    # replicated tensors (shared across cores)
    gT = np.ascontiguousarray(
        gate_w.T.reshape(KT, 128, E).transpose(1, 0, 2)).astype(NP_BF16)
    A_flat = lora_A.reshape(ER, D)
    aT = np.ascontiguousarray(
        A_flat.T.reshape(KT, 128, ER).transpose(1, 0, 2)).astype(NP_BF16)
    bT = np.ascontiguousarray(
        lora_B.transpose(0, 2, 1).reshape(ER, O)).astype(NP_BF16)
    Rm = np.repeat(np.eye(E, dtype=np.float32), R, axis=1).astype(NP_BF16)
    wT = np.ascontiguousarray(
        base_w.reshape(OTN, 128, KT, 128).transpose(3, 0, 2, 1)).astype(NP_BF16)
    bias2 = np.ascontiguousarray(base_b.reshape(OTN, 128).T)

    in_maps = []
    for c in range(N_CORES):
        x_c = xf[c * T:(c + 1) * T]                         # [T, D]
        xTc = np.ascontiguousarray(
            x_c.T.reshape(KT, 128, T).transpose(1, 0, 2)).astype(NP_BF16)
        in_maps.append({"xT": xTc, "wT": wT, "aT": aT, "gT": gT,
                        "bT": bT, "bias2": bias2, "Rm": Rm})
    return in_maps


def gather_outputs(results):
    """list of 8 per-core result maps -> FULL output [B, S, O]."""
    full = np.empty((B * S, O), dtype=np.float32)
    for c in range(N_CORES):
        oc = results[c]["out"]                              # [128, OTN, T] bf16
        full[c * T:(c + 1) * T, :] = \
            oc.transpose(2, 1, 0).reshape(T, O).astype(np.float32)
    return full.reshape(B, S, O)


_NC_CACHE = {}


def _get_module():
    if "nc" not in _NC_CACHE:
        _NC_CACHE["nc"] = build_module()
    return _NC_CACHE["nc"]


def run_sharded(in_maps, **run_kwargs):
    nc = _get_module()
    return run_bass_kernel_spmd(nc, in_maps, list(range(N_CORES)), **run_kwargs)


def kernel(x, gate_w, base_w, base_b, lora_A, lora_B):
    in_maps = shard_inputs(x, gate_w, base_w, base_b, lora_A, lora_B)
    res = run_sharded(in_maps)
    return gather_outputs(res.results)
